# revision 1
# baseline (speedup 1.0000x reference)
"""Sparse (conv-compressed) multi-head attention on 8 Trainium2 NeuronCores.

Entry point: kernel(**inputs) -> np.ndarray [4, 2048, 1024] float32.


Sharding: core c = 2*b + g  (b = batch 0..3, g = head-half 0..1).
Each core: batch b, heads [8g, 8g+8), all 2048 queries.
Final projection produces a partial (dv-half contraction); host sums pairs + bias.

Layout is fully transposed (channels on partitions):
  kT [1024, 2048] -> conv (as strided matmul, out-channel half, pair-AllGather)
  -> kcT [1024, 683] -> kpT [512, 683], vp_aug [683, 8*65] (ones col per head)
  qT [1024, 2048] -> qpT [512, 2048]
  per head h, q-tile m (512), j-chunk jc (128): sT = kpT_h.T @ qpT_h
  mask-add (host tiles) -> exp (ACT, scale=1/8) -> eT
  o_aug = vp_aug_h.T @ eT  (row 64 = softmax denom)
  o_n = o_aug[0:64] * (1/S broadcast via K=1 matmul)
  out_partial = o_nT.T @ WoT_half
"""
import sys
sys.path.insert(0, '/opt/trn_rl_repo')
import numpy as np
import concourse.bass as bass
import concourse.bacc as bacc
import concourse.mybir as mybir
from concourse import tile
from contextlib import ExitStack

f32 = mybir.dt.float32
f32r = mybir.dt.float32r
bf16 = mybir.dt.bfloat16
DT = bf16      # matmul operand dtype (all phases)
DT_ATT = bf16  # dtype for attention matmul operands (kpT/qpT/vp_aug/eT)
Exp = mybir.ActivationFunctionType.Exp

B, T, D, H = 4, 2048, 1024, 16
DH = 64
TC = 683          # compressed keys: 1 + 682
TCONV = 682
KK = 3 * D        # 3072 contraction for conv
DHALF = D // 2    # per-core head-half width
H8 = H // 2       # heads per core
NEG = -1.0e9
SCALE = DH ** -0.5  # 0.125

# conv N-splits (682 = 342 + 340, both >= 256 for f32r full rate)
CONV_NS = [(0, 342), (342, 340)]
# kp N-splits: fp32r matmul needs even N -> overlap col 341 (written twice)
KP_NS = [(0, 342), (341, 342)]

# attention q-tiles (4 x 512) and j-chunks (6 x 128, last = 43 rows)
NJ = 6
JROWS = [128, 128, 128, 128, 128, TC - 5 * 128]  # last = 43
JCS = {m: [jc for jc in range(NJ) if 384 * jc < 512 * (m + 1)] for m in range(4)}
# ragged tiles (m, jc) -> masked column count c1 (cols [0, c1) get mask add)
RAGGED = {}
for m in range(4):
    for jc in JCS[m]:
        pure = (384 * jc + 381 <= 512 * m)
        if not pure:
            RAGGED[(m, jc)] = min(512, 384 * jc + 381 - 512 * m)
C0 = {}
for m in range(4):
    for jc in JCS[m]:
        C0[(m, jc)] = max(0, 384 * jc - 512 * m)
RAGGED_LIST = sorted(RAGGED.keys())  # 8 tiles
assert len(RAGGED_LIST) == 8



def build_nc():
    nc = bacc.Bacc(None, target_bir_lowering=False, debug=False)

    qT = nc.dram_tensor("qT", [D, T], DT, kind="ExternalInput")
    kT = nc.dram_tensor("kT", [D, T], DT, kind="ExternalInput")
    WcH = nc.dram_tensor("WcH", [KK, DHALF], DT, kind="ExternalInput")
    WqT = nc.dram_tensor("WqT", [D, DHALF], DT, kind="ExternalInput")
    WkT = nc.dram_tensor("WkT", [D, DHALF], DT, kind="ExternalInput")
    WvT = nc.dram_tensor("WvT", [D, DHALF], DT, kind="ExternalInput")
    WoT = nc.dram_tensor("WoT", [DHALF, D], DT, kind="ExternalInput")
    MASK = nc.dram_tensor("MASK", [8, 128, 512], f32, kind="ExternalInput")
    K0H = nc.dram_tensor("K0H", [DHALF, 1], DT, kind="ExternalInput")
    SEL = nc.dram_tensor("SEL", [32, 2048], f32r, kind="ExternalInput")  # bcast selector
    OUT = nc.dram_tensor("out_p", [T, D], f32, kind="ExternalOutput")

    kc_half = nc.dram_tensor("kc_half", [DHALF, TC], DT)
    kc_full = nc.dram_tensor("kc_full", [D, TC], DT)

    with tile.TileContext(nc) as tc, ExitStack() as st:
        st.enter_context(nc.allow_low_precision("float32r is 4-byte fp32 storage"))
        pool = lambda **kw: st.enter_context(tc.tile_pool(**kw))
        p_big = pool(name="big", bufs=8)        # kT -> qpT + o_nT [128,2048]
        p_ws = pool(name="wstream", bufs=4)     # streamed weights [128,<=512]
        p_kc = pool(name="kc", bufs=8)          # kcT [128,683]
        p_kp = pool(name="kp", bufs=4)          # kpT [128,683]
        p_vpa = pool(name="vpa", bufs=6)        # vp_aug [128,520]
        p_mask = pool(name="mask", bufs=8)      # mask tiles [128,512] f32
        p_qt = pool(name="qt", bufs=12)         # qT slices [128,512]
        p_et = pool(name="et", bufs=8)          # eT [128,512]
        p_out = pool(name="outsb", bufs=3)      # out staging [128,1024] f32
        p_wo = pool(name="wo", bufs=8)          # WoT resident [128,512]
        p_small = pool(name="small", bufs=4)
        p_ont = pool(name="ont", bufs=16)       # o_nT split per (dv-chunk, m) [128,512]

        kt = [None] * 8

        def load_kt(ic):
            t = p_big.tile([128, T], DT, name="big", tag="big")
            nc.sync.dma_start(t[:], kT[128 * ic:128 * (ic + 1), :])
            kt[ic] = t

        # ---- conv: own out-channel half, K-contiguous, WcH streamed once ----
        with tc.tile_pool(name="ps_conv", bufs=8, space="PSUM") as ps_conv:
            ps_kc = {}
            for kk in range(24):
                if kk < 8:
                    load_kt(kk)
                wc_t = p_ws.tile([128, DHALF], DT, name="ws", tag="ws")
                nc.sync.dma_start(wc_t[:], WcH[128 * kk:128 * (kk + 1), :])
                r, ic = kk // 8, kk % 8
                for ni, (t0, tw) in enumerate(CONV_NS):
                    rhs = kt[ic][:, :3 * TCONV].rearrange(
                        "p (t r) -> p t r", r=3)[:, t0:t0 + tw, r]
                    for mc in range(4):
                        if kk == 0:
                            ps_kc[(ni, mc)] = ps_conv.tile(
                                [128, 342], f32, name="ps_kc", tag="ps_kc")
                        nc.tensor.matmul(
                            ps_kc[(ni, mc)][:, :tw],
                            wc_t[:, 128 * mc:128 * (mc + 1)],
                            rhs,
                            start=(kk == 0), stop=(kk == 23))
            for mc in range(4):
                t = p_kc.tile([128, TC], DT, name="kc", tag="kc")
                for ni, (t0, tw) in enumerate(CONV_NS):
                    nc.vector.tensor_copy(
                        t[:, 1 + t0:1 + t0 + tw], ps_kc[(ni, mc)][:, :tw])
                nc.sync.dma_start(kc_half[128 * mc:128 * (mc + 1), 1:], t[:, 1:])
        with nc.allow_non_contiguous_dma(reason="512x1 col write, 2KB total"):
            nc.sync.dma_start(kc_half[:, 0:1], K0H[:])

        nc.gpsimd.collective_compute(
            "AllGather", mybir.AluOpType.bypass,
            replica_groups=[[0, 1], [2, 3], [4, 5], [6, 7]],
            ins=[kc_half[:]], outs=[kc_full[:]],
        )

        kc = []
        for c in range(8):
            t = p_kc.tile([128, TC], DT, name="kc", tag="kc")
            nc.sync.dma_start(t[:], kc_full[128 * c:128 * (c + 1), :])
            kc.append(t)

        # ---- qp^T = WqT-lhsT @ qT  [512, 2048], 2 passes of 2 n-tiles ----
        qpt = [p_big.tile([128, T], DT_ATT, name="big", tag="big") for _ in range(4)]
        with tc.tile_pool(name="ps_qp", bufs=8, space="PSUM") as ps_pool:
            for npass in range(2):
                ps_qp = {}
                for kk in range(8):
                    wq_t = p_ws.tile([128, DHALF], DT, name="ws", tag="ws")
                    nc.sync.dma_start(wq_t[:], WqT[128 * kk:128 * (kk + 1), :])
                    for n in (2 * npass, 2 * npass + 1):
                        qt_t = p_qt.tile([128, 512], DT, name="qt", tag="qt")
                        nc.sync.dma_start(
                            qt_t[:], qT[128 * kk:128 * (kk + 1), 512 * n:512 * (n + 1)])
                        for m in range(4):
                            if kk == 0:
                                ps_qp[(m, n)] = ps_pool.tile(
                                    [128, 512], f32, name="ps_qp", tag="ps_qp")
                            nc.tensor.matmul(
                                ps_qp[(m, n)][:],
                                wq_t[:, 128 * m:128 * (m + 1)],
                                qt_t[:],
                                start=(kk == 0), stop=(kk == 7))
                for (m, n), ps in ps_qp.items():
                    nc.vector.tensor_copy(qpt[m][:, 512 * n:512 * (n + 1)], ps[:])

        # ---- vp_aug [683, 8*65]: vp = kcT-lhsT @ WvT, + ones columns ----
        ones_vpa = p_small.tile([128, 8], f32, name="ones_vpa", tag="ones_vpa", bufs=1)
        nc.vector.memset(ones_vpa[:], 1.0)
        vpa = [p_vpa.tile([128, H8 * 65], DT_ATT, name="vpa", tag="vpa")
               for _ in range(NJ)]
        with tc.tile_pool(name="ps_vp", bufs=6, space="PSUM") as ps_pool:
            ps_vp = {}
            for kk in range(8):
                wv_t = p_ws.tile([128, DHALF], DT, name="ws", tag="ws")
                nc.sync.dma_start(wv_t[:], WvT[128 * kk:128 * (kk + 1), :])
                for jb in range(NJ):
                    jr = JROWS[jb]
                    if kk == 0:
                        ps_vp[jb] = ps_pool.tile(
                            [128, 512], f32, name="ps_vp", tag="ps_vp")
                    nc.tensor.matmul(
                        ps_vp[jb][:jr, :],
                        kc[kk][:, 128 * jb:128 * jb + jr],
                        wv_t[:],
                        start=(kk == 0), stop=(kk == 7))
            for jb in range(NJ):
                jr = JROWS[jb]
                dst = vpa[jb][:jr, :].rearrange("p (h c) -> p h c", c=65)
                src = ps_vp[jb][:jr, :].rearrange("p (h c) -> p h c", c=64)
                nc.vector.tensor_copy(dst[:, :, 0:64], src[:])
                nc.vector.tensor_copy(
                    dst[:, :, 64:65],
                    ones_vpa[:jr, :].rearrange("p (h c) -> p h c", c=1))

        # ---- kp^T = WkT-lhsT @ kcT  [512, 683] ----
        kpt = [p_kp.tile([128, TC], DT_ATT, name="kp", tag="kp") for _ in range(4)]
        with tc.tile_pool(name="ps_kp", bufs=8, space="PSUM") as ps_pool:
            ps_kp = {}
            for kk in range(8):
                wk_t = p_ws.tile([128, DHALF], DT, name="ws", tag="ws")
                nc.sync.dma_start(wk_t[:], WkT[128 * kk:128 * (kk + 1), :])
                for m in range(4):
                    for ni, (t0, tw) in enumerate(KP_NS):
                        if kk == 0:
                            ps_kp[(m, ni)] = ps_pool.tile(
                                [128, 342], f32, name="ps_kp", tag="ps_kp")
                        nc.tensor.matmul(
                            ps_kp[(m, ni)][:, :tw],
                            wk_t[:, 128 * m:128 * (m + 1)],
                            kc[kk][:, t0:t0 + tw],
                            start=(kk == 0), stop=(kk == 7))
            for (m, ni), ps in ps_kp.items():
                t0, tw = KP_NS[ni]
                nc.vector.tensor_copy(kpt[m][:, t0:t0 + tw], ps[:, :tw])

        # ---- masks + constants ----
        mk = []
        for t_i in range(8):
            mt = p_mask.tile([128, 512], f32, name="mask", tag="mask")
            nc.sync.dma_start(mt[:], MASK[t_i])
            mk.append(mt)
        sel = p_small.tile([32, 2048], f32r, name="sel", tag="sel", bufs=1)
        nc.sync.dma_start(sel[:], SEL[:])

        # ---- attention (h outer, jc mid for lhsT reuse, m inner) ----
        # o_nt holds UNNORMALIZED head outputs; S_all collects denominators.
        o_nt = {(kk, m): p_ont.tile([128, 512], DT, name="ont", tag="ont")
                for kk in range(4) for m in range(4)}
        S_all = p_small.tile([32, 512], f32, name="s_all", tag="s_all", bufs=1)
        with tc.tile_pool(name="ps_att", bufs=3, space="PSUM") as ps_att:
            for h in range(H8):
                hc, off = h // 2, (h % 2) * 64
                po = {m: ps_att.tile([128, 512], f32, name="ps_o", tag="ps_o", bufs=4)
                      for m in range(4)}
                for jc in range(NJ):
                    jr = JROWS[jc]
                    for m in range(4):
                        if jc not in JCS[m]:
                            continue
                        key = (m, jc)
                        c0 = C0[key]
                        ps = ps_att.tile([128, 512], f32, name="ps_s", tag="ps_s", bufs=4)
                        nc.tensor.matmul(
                            ps[:jr, c0:],
                            kpt[hc][off:off + 64, 128 * jc:128 * jc + jr],
                            qpt[hc][off:off + 64, 512 * m + c0:512 * (m + 1)],
                            start=True, stop=True)
                        if key in RAGGED:
                            c1 = RAGGED[key]
                            ti = RAGGED_LIST.index(key)
                            nc.vector.tensor_add(
                                ps[:jr, c0:c1], ps[:jr, c0:c1], mk[ti][:jr, c0:c1])
                        et = p_et.tile([128, 512], DT_ATT, name="et", tag="et")
                        nc.scalar.activation(et[:jr, c0:], ps[:jr, c0:], Exp, scale=SCALE)
                        nc.tensor.matmul(
                            po[m][:65, c0:],
                            vpa[jc][:jr, 65 * h:65 * (h + 1)],
                            et[:jr, c0:],
                            start=(jc == 0), stop=(jc == JCS[m][-1]))
                for m in range(4):
                    idx = m * 8 + h
                    s_stage = p_small.tile([1, 512], f32, name="s_stage",
                                           tag="s_stage", bufs=4)
                    nc.vector.tensor_copy(s_stage[:], po[m][64:65, :])
                    nc.sync.dma_start(S_all[idx:idx + 1, :], s_stage[:])
                    nc.vector.tensor_copy(
                        o_nt[(hc, m)][off:off + 64, :], po[m][0:64, :])

        # ---- normalize + final, interleaved by m in ONE PSUM pool so the
        # final matmuls overlap the remaining broadcast/mult work ----
        R_all = p_small.tile([32, 512], f32r, name="r_all", tag="r_all", bufs=1)
        nc.vector.reciprocal(R_all[:], S_all[:])
        wot = {}
        for kk in range(4):
            for nn in range(2):
                wt = p_wo.tile([128, 512], DT, name="wo", tag="wo")
                nc.sync.dma_start(
                    wt[:], WoT[128 * kk:128 * (kk + 1), 512 * nn:512 * (nn + 1)])
                wot[(kk, nn)] = wt
        with tc.tile_pool(name="ps_nf", bufs=3, space="PSUM") as ps_nf:
            for m in range(4):
                for p in range(4 * m, 4 * m + 4):
                    bc = ps_nf.tile([128, 512], f32, name="ps_bc", tag="ps_bc", bufs=3)
                    nc.tensor.matmul(
                        bc[:], sel[:, 128 * p:128 * (p + 1)], R_all[:],
                        start=True, stop=True)
                    for half in range(2):
                        idx = 2 * p + half
                        mm, h = divmod(idx, 8)
                        hc, off = h // 2, (h % 2) * 64
                        dst = o_nt[(hc, mm)][off:off + 64, :]
                        nc.vector.tensor_mul(dst, dst, bc[64 * half:64 * half + 64, :])
                for mq in range(4 * m, 4 * m + 4):
                    ob = p_out.tile([128, D], f32, name="outsb", tag="outsb")
                    for nn in range(2):
                        pf = ps_nf.tile([128, 512], f32, name="ps_f", tag="ps_f", bufs=4)
                        for kk in range(4):
                            nc.tensor.matmul(
                                pf[:],
                                o_nt[(kk, mq // 4)][:, 128 * (mq % 4):128 * (mq % 4 + 1)],
                                wot[(kk, nn)][:],
                                start=(kk == 0), stop=(kk == 3))
                        nc.vector.tensor_copy(ob[:, 512 * nn:512 * (nn + 1)], pf[:])
                    nc.sync.dma_start(OUT[128 * mq:128 * (mq + 1), :], ob[:])

    return nc


def make_mask() -> np.ndarray:
    mask = np.zeros((8, 128, 512), dtype=np.float32)
    for t, (m, jc) in enumerate(RAGGED_LIST):
        q = 512 * m + np.arange(512)[None, :]
        j = 128 * jc + np.arange(128)[:, None]
        mask[t] = np.where(3 * j > q, NEG, 0.0).astype(np.float32)
    return mask


def make_sel() -> np.ndarray:
    m = np.arange(2048)
    k_of_m = 2 * (m // 128) + (m % 128) // 64
    sel = (np.arange(32)[:, None] == k_of_m[None, :]).astype(np.float32)
    return sel


def prep_inputs(q, k, Wq, Wk, Wv, Wo, conv_w):
    """Returns list of 8 in_maps (core c = 2b + g)."""
    import ml_dtypes
    bf = ml_dtypes.bfloat16
    Wc = np.ascontiguousarray(conv_w.transpose(2, 1, 0).reshape(KK, D))
    mask = make_mask()
    sel = make_sel()
    in_maps = []
    for c in range(8):
        b, g = c // 2, c % 2
        sl = slice(DHALF * g, DHALF * (g + 1))
        in_maps.append({
            "qT": np.ascontiguousarray(q[b].T).astype(bf),
            "kT": np.ascontiguousarray(k[b].T).astype(bf),
            "WcH": np.ascontiguousarray(Wc[:, sl]).astype(bf),
            "WqT": np.ascontiguousarray(Wq[sl, :].T).astype(bf),
            "WkT": np.ascontiguousarray(Wk[sl, :].T).astype(bf),
            "WvT": np.ascontiguousarray(Wv[sl, :].T).astype(bf),
            "WoT": np.ascontiguousarray(Wo[:, sl].T).astype(bf),
            "MASK": mask,
            "K0H": np.ascontiguousarray(k[b, 0, sl].reshape(DHALF, 1)).astype(bf),
            "SEL": sel,
        })
    return in_maps


def postprocess(results, bo):
    out = np.zeros((B, T, D), dtype=np.float32)
    for b in range(B):
        out[b] = (np.asarray(results[2 * b]["out_p"], dtype=np.float32)
                  + np.asarray(results[2 * b + 1]["out_p"], dtype=np.float32)
                  + bo[None, :])
    return out


_CACHED_NC = None


def kernel(q, k, v, Wq, Wk, Wv, Wo, bo, conv_w):
    """Full-input entry point. v is unused by the reference computation
    (V is replaced by the conv-compressed K)."""
    global _CACHED_NC
    from concourse.bass_utils import run_bass_kernel_spmd

    q = np.asarray(q, dtype=np.float32)
    k = np.asarray(k, dtype=np.float32)
    Wq = np.asarray(Wq, dtype=np.float32)
    Wk = np.asarray(Wk, dtype=np.float32)
    Wv = np.asarray(Wv, dtype=np.float32)
    Wo = np.asarray(Wo, dtype=np.float32)
    bo = np.asarray(bo, dtype=np.float32)
    conv_w = np.asarray(conv_w, dtype=np.float32)

    in_maps = prep_inputs(q, k, Wq, Wk, Wv, Wo, conv_w)
    if _CACHED_NC is None:
        nc = build_nc()
        nc.finalize()
        _CACHED_NC = nc
    res = run_bass_kernel_spmd(_CACHED_NC, in_maps, list(range(8)))
    return postprocess(res.results, bo)



# revision 23
# speedup vs baseline: 1.1068x; 1.1068x over previous
"""Sparse (conv-compressed) multi-head attention on 8 Trainium2 NeuronCores.

Entry point: kernel(**inputs) -> np.ndarray [4, 2048, 1024] float32.

Sharding: core c = 2*b + g  (b = batch 0..3, g = head-half 0..1).
Each core: batch b, heads [8g, 8g+8), all 2048 queries.
Final projection produces a partial (dv-half contraction); host sums pairs + bias.

v2 scheduling (target: PE warm at 2.4 GHz end-to-end):
- conv is mc-outer so the kc exchange can start early; the pair exchange is a
  ReduceScatter with per-core 0/1 slot scales (SAB input), which delivers ONLY
  the peer half at a fixed dram address (SPMD-safe) while the own half stays
  SBUF-resident from the conv.
- the collective runs concurrently with the qp projection; kp/vp contract the
  4 local chunks first so the exchange latency is hidden.
- attention is m-outer / head-pair inner, scores pipelined one step ahead of
  attn@V, with the normalize + final-projection work of m-1 interleaved into
  the attention PE stream of m (filler units) so the PE never idles (HAM
  stays un-throttled) and the scalar-engine exp stream is overlapped.
"""
import sys
sys.path.insert(0, '/opt/trn_rl_repo')
import numpy as np
import concourse.bass as bass
import concourse.bacc as bacc
import concourse.mybir as mybir
from concourse import tile
from contextlib import ExitStack

f32 = mybir.dt.float32
f32r = mybir.dt.float32r
bf16 = mybir.dt.bfloat16
DT = bf16
Exp = mybir.ActivationFunctionType.Exp

B, T, D, H = 4, 2048, 1024, 16
DH = 64
TC = 683          # compressed keys: 1 + 682
TCONV = 682
KK = 3 * D        # 3072 contraction for conv
DHALF = D // 2    # per-core head-half width
H8 = H // 2       # heads per core
NEG = -1.0e9
SCALE = DH ** -0.5  # 0.125

CONV_NS = [(0, 342), (342, 340)]
KP_NS = [(0, 342), (342, 341)]
USE_RS = False  # ReduceScatter peer-exchange vs plain AllGather (must match host prep)

# attention q-tiles (4 x 512) and j-chunks (6 x 128, last = 43 rows)
NJ = 6
JROWS = [128, 128, 128, 128, 128, TC - 5 * 128]  # last = 43
JCS = {m: [jc for jc in range(NJ) if 384 * jc < 512 * (m + 1)] for m in range(4)}
RAGGED = {}
for m in range(4):
    for jc in JCS[m]:
        pure = (384 * jc + 381 <= 512 * m)
        if not pure:
            RAGGED[(m, jc)] = min(512, 384 * jc + 381 - 512 * m)
C0 = {}
for m in range(4):
    for jc in JCS[m]:
        C0[(m, jc)] = max(0, 384 * jc - 512 * m)
RAGGED_LIST = sorted(RAGGED.keys())  # 8 tiles
assert len(RAGGED_LIST) == 8

# conv contraction order: ic-major so each kT chunk unlocks 3 steps
KK_ORDER = [r * 8 + ic for ic in range(8) for r in range(3)]


def build_nc():
    nc = bacc.Bacc(None, target_bir_lowering=False, debug=False)

    qT = nc.dram_tensor("qT", [D, T], DT, kind="ExternalInput")
    kT = nc.dram_tensor("kT", [D, T], DT, kind="ExternalInput")
    WcH = nc.dram_tensor("WcH", [KK, DHALF], DT, kind="ExternalInput")
    WqT = nc.dram_tensor("WqT", [D, DHALF], DT, kind="ExternalInput")
    WkT = nc.dram_tensor("WkT", [D, DHALF], DT, kind="ExternalInput")  # host row-permuted [own; peer]
    WvT = nc.dram_tensor("WvT", [D, DHALF], DT, kind="ExternalInput")  # host row-permuted [own; peer]
    WoT = nc.dram_tensor("WoT", [DHALF, D], DT, kind="ExternalInput")
    MASK = nc.dram_tensor("MASK", [8, 128, 512], DT, kind="ExternalInput")
    K0H = nc.dram_tensor("K0H", [DHALF, 1], DT, kind="ExternalInput")
    SAB = nc.dram_tensor("SAB", [128, 2], f32, kind="ExternalInput")  # col0=g, col1=1-g
    SEL = nc.dram_tensor("SEL", [32, 2048], f32r, kind="ExternalInput")
    OUT = nc.dram_tensor("out_p", [T, D], f32, kind="ExternalOutput")

    if USE_RS:
        kc_x2 = nc.dram_tensor("kc_x2", [D, TC], DT)  # [own*sA ; own*sB]
        kc_peer = nc.dram_tensor("kc_peer", [DHALF, TC], DT)
    else:
        kc_half = nc.dram_tensor("kc_half", [DHALF, TC], DT)
        kc_full = nc.dram_tensor("kc_full", [D, TC], DT)

    with tile.TileContext(nc) as tc, ExitStack() as st:
        st.enter_context(nc.allow_low_precision("float32r is 4-byte fp32 storage"))
        pool = lambda **kw: st.enter_context(tc.tile_pool(**kw))
        p_big = pool(name="big", bufs=8)        # kT x8 then qpt x4 (ring reuse)
        p_qt = pool(name="qt", bufs=32)         # qT fully resident [128,512]
        p_wc = pool(name="wc", bufs=12)         # conv weight chunks [128,128]
        p_wq = pool(name="wq", bufs=8)
        p_wk = pool(name="wk", bufs=8)
        p_wv = pool(name="wv", bufs=8)
        p_wo = pool(name="wo", bufs=4)          # [128,1024]
        p_kco = pool(name="kco", bufs=4)        # own conv half [128,683]
        p_kcp = pool(name="kcp", bufs=4 if USE_RS else 8)  # peer/full kc [128,683]
        p_stage = pool(name="stage", bufs=4)    # scaled staging [128,683]
        p_kp = pool(name="kp", bufs=4)          # kpT [128,683]
        p_vpa = pool(name="vpa", bufs=6)        # vp_aug [128,520]
        p_mask = pool(name="mask", bufs=8)      # mask tiles [128,512] bf16
        p_et = pool(name="et", bufs=6)          # eT [128,512]
        p_ont = pool(name="ont", bufs=16)       # o_nT [128,512]
        p_out = pool(name="outsb", bufs=4)      # out staging [128,1024] f32
        p_small = pool(name="small", bufs=4)

        # ---- prefetch streams ----
        # sync stream: conv-critical (kT, WcH chunks inline below)
        kt = []
        for ic in range(8):
            t = p_big.tile([128, T], DT, name="big", tag="big")
            nc.sync.dma_start(t[:], kT[128 * ic:128 * (ic + 1), :])
            kt.append(t)
        sab = p_small.tile([128, 2], f32, name="sab", tag="sab", bufs=1)
        nc.sync.dma_start(sab[:], SAB[:])

        # prefetch closures, trickled on the sync FIFO between conv steps so
        # the wc ring paces them and conv-critical DMA keeps priority
        qt = {}
        wq, wk, wv, mk, wo = [], [], [], [], []
        sel = p_small.tile([32, 2048], f32r, name="sel", tag="sel", bufs=1)
        _trickle = []

        def _ld(dst_list_or_map, key, pool_, shape, src, tag):
            def go():
                t = pool_.tile(shape, DT, name=tag, tag=tag)
                nc.sync.dma_start(t[:], src)
                if key is None:
                    dst_list_or_map.append(t)
                else:
                    dst_list_or_map[key] = t
            _trickle.append(go)

        for n in range(4):
            for kk in range(8):
                _ld(qt, (kk, n), p_qt, [128, 512],
                    qT[128 * kk:128 * (kk + 1), 512 * n:512 * (n + 1)], "qt")
        for kk in range(8):
            _ld(wq, None, p_wq, [128, DHALF], WqT[128 * kk:128 * (kk + 1), :], "wq")
        for kk in range(8):
            _ld(wk, None, p_wk, [128, DHALF], WkT[128 * kk:128 * (kk + 1), :], "wk")
        for kk in range(8):
            _ld(wv, None, p_wv, [128, DHALF], WvT[128 * kk:128 * (kk + 1), :], "wv")
        for t_i in range(8):
            _ld(mk, None, p_mask, [128, 512], MASK[t_i], "mask")
        _trickle.append(lambda: nc.sync.dma_start(sel[:], SEL[:]))
        for kk in range(4):
            _ld(wo, None, p_wo, [128, D], WoT[128 * kk:128 * (kk + 1), :], "wo")

        ones_vpa = p_small.tile([128, 8], f32, name="ones_vpa", tag="ones_vpa", bufs=1)
        nc.vector.memset(ones_vpa[:], 1.0)
        S_all = p_small.tile([32, 512], f32, name="s_all", tag="s_all", bufs=1)
        R_scr = p_small.tile([32, 512], f32, name="r_scr", tag="r_scr", bufs=1)
        R_all = p_small.tile([32, 512], f32r, name="r_all", tag="r_all", bufs=1)
        # recip runs over all 32 partitions (aligned base); unwritten S rows
        # must stay finite so sel-zeros don't turn into 0*inf = NaN
        nc.vector.memset(S_all[:], 1.0)

        # ---- conv: kk-outer (8 psum banks), prefetch trickled behind the
        # wc ring so conv-critical DMA gets priority bandwidth ----
        trickle = list(_trickle)
        kc_own = []
        with tc.tile_pool(name="ps_conv", bufs=8, space="PSUM") as ps_conv:
            ps_kc = {}
            for step, kk in enumerate(KK_ORDER):
                r, ic = kk // 8, kk % 8
                wc_t = p_wc.tile([128, DHALF], DT, name="wc", tag="wc")
                nc.sync.dma_start(wc_t[:], WcH[128 * kk:128 * (kk + 1), :])
                for ni, (t0, tw) in enumerate(CONV_NS):
                    rhs = kt[ic][:, :3 * TCONV].rearrange(
                        "p (t r) -> p t r", r=3)[:, t0:t0 + tw, r]
                    for mc in range(4):
                        if step == 0:
                            ps_kc[(ni, mc)] = ps_conv.tile(
                                [128, 342], f32, name="ps_kc", tag="ps_kc")
                        nc.tensor.matmul(
                            ps_kc[(ni, mc)][:, :tw],
                            wc_t[:, 128 * mc:128 * (mc + 1)], rhs,
                            start=(step == 0), stop=(step == 23))
                for _ in range(1 if step < 8 else 3):
                    if trickle:
                        trickle.pop(0)()
            while trickle:
                trickle.pop(0)()
            for mc in range(4):
                t = p_kco.tile([128, TC], DT, name="kco", tag="kco")
                with nc.allow_non_contiguous_dma(reason="128x1 col write, tiny"):
                    nc.sync.dma_start(t[:, 0:1], K0H[128 * mc:128 * (mc + 1), :])
                for ni, (t0, tw) in enumerate(CONV_NS):
                    nc.vector.tensor_copy(
                        t[:, 1 + t0:1 + t0 + tw], ps_kc[(ni, mc)][:, :tw])
                kc_own.append(t)
                if USE_RS:
                    for sl in range(2):
                        st_t = p_stage.tile([128, TC], DT, name="stage", tag="stage")
                        nc.scalar.mul(st_t[:], t[:], sab[:, sl:sl + 1])
                        nc.sync.dma_start(
                            kc_x2[DHALF * sl + 128 * mc:DHALF * sl + 128 * (mc + 1), :],
                            st_t[:])
                else:
                    nc.sync.dma_start(
                        kc_half[128 * mc:128 * (mc + 1), :], t[:])

        if USE_RS:
            nc.gpsimd.collective_compute(
                "ReduceScatter", mybir.AluOpType.add,
                replica_groups=[[0, 1], [2, 3], [4, 5], [6, 7]],
                ins=[kc_x2[:]], outs=[kc_peer[:]],
            )
            kc_p = []
            for c in range(4):
                t = p_kcp.tile([128, TC], DT, name="kcp", tag="kcp")
                nc.sync.dma_start(t[:], kc_peer[128 * c:128 * (c + 1), :])
                kc_p.append(t)
            kcs = kc_own + kc_p  # contraction chunks [own x4, peer x4]
        else:
            nc.gpsimd.collective_compute(
                "AllGather", mybir.AluOpType.bypass,
                replica_groups=[[0, 1], [2, 3], [4, 5], [6, 7]],
                ins=[kc_half[:]], outs=[kc_full[:]],
            )
            kcs = []
            for c in range(8):
                t = p_kcp.tile([128, TC], DT, name="kcp", tag="kcp")
                nc.sync.dma_start(t[:], kc_full[128 * c:128 * (c + 1), :])
                kcs.append(t)

        # ---- qp^T = WqT-lhsT @ qT  [512, 2048]; runs while the RS is in flight ----
        qpt = [p_big.tile([128, T], DT, name="big", tag="big") for _ in range(4)]
        with tc.tile_pool(name="ps_qp", bufs=8, space="PSUM") as ps_pool:
            for npass in range(2):
                ps_qp = {}
                for kk in range(8):
                    for n in (2 * npass, 2 * npass + 1):
                        for m in range(4):
                            if kk == 0:
                                ps_qp[(m, n)] = ps_pool.tile(
                                    [128, 512], f32, name="ps_qp", tag="ps_qp")
                            nc.tensor.matmul(
                                ps_qp[(m, n)][:],
                                wq[kk][:, 128 * m:128 * (m + 1)],
                                qt[(kk, n)][:],
                                start=(kk == 0), stop=(kk == 7))
                for (m, n), psq in ps_qp.items():
                    nc.scalar.copy(qpt[m][:, 512 * n:512 * (n + 1)], psq[:])

        # ---- kp^T = WkT-lhsT @ kcT  [512, 683]; local chunks first ----
        kpt = [p_kp.tile([128, TC], DT, name="kp", tag="kp") for _ in range(4)]
        with tc.tile_pool(name="ps_kp", bufs=8, space="PSUM") as ps_pool:
            ps_kp = {}
            for kk in range(8):
                for m in range(4):
                    for ni, (t0, tw) in enumerate(KP_NS):
                        if kk == 0:
                            ps_kp[(m, ni)] = ps_pool.tile(
                                [128, 342], f32, name="ps_kp", tag="ps_kp")
                        nc.tensor.matmul(
                            ps_kp[(m, ni)][:, :tw],
                            wk[kk][:, 128 * m:128 * (m + 1)],
                            kcs[kk][:, t0:t0 + tw],
                            start=(kk == 0), stop=(kk == 7))
            for (m, ni), psk in ps_kp.items():
                t0, tw = KP_NS[ni]
                nc.vector.tensor_copy(kpt[m][:, t0:t0 + tw], psk[:, :tw])

        # ---- vp_aug [683, 8*65]: vp = kcT-lhsT @ WvT, + ones columns ----
        vpa = [p_vpa.tile([128, H8 * 65], DT, name="vpa", tag="vpa")
               for _ in range(NJ)]
        with tc.tile_pool(name="ps_vp", bufs=6, space="PSUM") as ps_pool:
            ps_vp = {}
            for kk in range(8):
                for jb in range(NJ):
                    jr = JROWS[jb]
                    if kk == 0:
                        ps_vp[jb] = ps_pool.tile(
                            [128, 512], f32, name="ps_vp", tag="ps_vp")
                    nc.tensor.matmul(
                        ps_vp[jb][:jr, :],
                        kcs[kk][:, 128 * jb:128 * jb + jr],
                        wv[kk][:],
                        start=(kk == 0), stop=(kk == 7))
            for jb in range(NJ):
                jr = JROWS[jb]
                dst = vpa[jb][:jr, :].rearrange("p (h c) -> p h c", c=65)
                src = ps_vp[jb][:jr, :].rearrange("p (h c) -> p h c", c=64)
                nc.vector.tensor_copy(dst[:, :, 0:64], src[:])
                nc.vector.tensor_copy(
                    dst[:, :, 64:65],
                    ones_vpa[:jr, :].rearrange("p (h c) -> p h c", c=1))

        # ---- attention (m outer, head-pair inner) + interleaved finals ----
        o_nt = {(kk, m): p_ont.tile([128, 512], DT, name="ont", tag="ont")
                for kk in range(4) for m in range(4)}
        ob_tiles = {}

        with tc.tile_pool(name="ps_att", bufs=4, space="PSUM") as ps_att, \
                tc.tile_pool(name="ps_po", bufs=2, space="PSUM") as ps_po, \
                tc.tile_pool(name="ps_fb", bufs=2, space="PSUM") as ps_fb:

            def emit_av(m, hp, po, item):
                hh, jc, et, jr, c0 = item
                h = 2 * hp + hh
                jcs = JCS[m]
                nc.tensor.matmul(
                    po[hh][:65, c0:],
                    vpa[jc][:jr, 65 * h:65 * (h + 1)],
                    et[:jr, c0:],
                    start=(jc == jcs[0]), stop=(jc == jcs[-1]))

            def make_fillers(m):
                ops = []

                for p in range(4 * m, 4 * m + 4):
                    def bc_unit(p=p):
                        bcp = ps_fb.tile([128, 512], f32, name="ps_bc", tag="ps_fb")
                        nc.tensor.matmul(
                            bcp[:], sel[:, 128 * p:128 * (p + 1)], R_all[:],
                            start=True, stop=True)
                        for half in range(2):
                            idx = 2 * p + half
                            mm_, h = divmod(idx, 8)
                            hc, off = h // 2, (h % 2) * 64
                            dst = o_nt[(hc, mm_)][off:off + 64, :]
                            nc.vector.tensor_mul(
                                dst, dst, bcp[64 * half:64 * half + 64, :])
                    ops.append(bc_unit)
                for mq in range(4 * m, 4 * m + 4):
                    for nn in range(2):
                        def pf_unit(mq=mq, nn=nn, m=m):
                            pf = ps_fb.tile([128, 512], f32, name="ps_f", tag="ps_fb")
                            for kk in range(4):
                                nc.tensor.matmul(
                                    pf[:],
                                    o_nt[(kk, m)][:, 128 * (mq - 4 * m):128 * (mq - 4 * m + 1)],
                                    wo[kk][:, 512 * nn:512 * (nn + 1)],
                                    start=(kk == 0), stop=(kk == 3))
                            if nn == 0:
                                ob_tiles[mq] = p_out.tile(
                                    [128, D], f32, name="outsb", tag="outsb")
                            ob = ob_tiles[mq]
                            nc.vector.tensor_copy(ob[:, 512 * nn:512 * (nn + 1)], pf[:])
                            nc.gpsimd.dma_start(
                                OUT[128 * mq:128 * (mq + 1), 512 * nn:512 * (nn + 1)],
                                ob[:, 512 * nn:512 * (nn + 1)])
                        ops.append(pf_unit)
                return ops

            fillers = []
            for m in range(4):
                jcs = JCS[m]
                n_groups = 4 * len(jcs)
                rate = len(fillers) / n_groups
                fcredit = 0.0
                for hp in range(4):
                    po = [ps_po.tile([128, 512], f32, name="ps_o", tag="ps_o")
                          for _ in range(2)]
                    pend = []
                    for jc in jcs:
                        jr = JROWS[jc]
                        c0 = C0[(m, jc)]
                        for hh in range(2):
                            h = 2 * hp + hh
                            off = 64 * hh
                            psx = ps_att.tile([128, 512], f32, name="ps_s", tag="ps_s")
                            nc.tensor.matmul(
                                psx[:jr, c0:],
                                kpt[hp][off:off + 64, 128 * jc:128 * jc + jr],
                                qpt[hp][off:off + 64, 512 * m + c0:512 * (m + 1)],
                                start=True, stop=True)
                            et = p_et.tile([128, 512], DT, name="et", tag="et")
                            nc.scalar.activation(
                                et[:jr, c0:], psx[:jr, c0:], Exp, scale=SCALE)
                            key = (m, jc)
                            if key in RAGGED:
                                # 0/1 mask multiply post-exp (bf16, 2x DVE rate)
                                c1 = RAGGED[key]
                                ti = RAGGED_LIST.index(key)
                                nc.vector.tensor_mul(
                                    et[:jr, c0:c1], et[:jr, c0:c1],
                                    mk[ti][:jr, c0:c1])
                            pend.append((hh, jc, et, jr, c0))
                        while len(pend) > 2:
                            emit_av(m, hp, po, pend.pop(0))
                        fcredit += rate
                        while fcredit >= 1.0 and fillers:
                            fillers.pop(0)()
                            fcredit -= 1.0
                    while pend:
                        emit_av(m, hp, po, pend.pop(0))
                    for hh in range(2):
                        h = 2 * hp + hh
                        nc.vector.tensor_copy(
                            o_nt[(hp, m)][64 * hh:64 * hh + 64, :], po[hh][0:64, :])
                        sst = p_small.tile([1, 512], f32, name="sst", tag="sst", bufs=4)
                        nc.vector.tensor_copy(sst[:], po[hh][64:65, :])
                        nc.sync.dma_start(
                            S_all[8 * m + h:8 * m + h + 1, :], sst[:])
                while fillers:
                    fillers.pop(0)()
                # reciprocal issued eagerly at m end (DVE), not as a filler,
                # so the first bc matmul of the next m never head-of-line
                # blocks the PE on it
                nc.vector.reciprocal_approx_fast(R_scr[:], S_all[:])
                nc.vector.tensor_copy(R_all[:], R_scr[:])
                fillers = make_fillers(m)
            while fillers:
                fillers.pop(0)()

    return nc


def make_mask() -> np.ndarray:
    """0/1 keep-mask, multiplied into exp(scores) post-activation."""
    mask = np.zeros((8, 128, 512), dtype=np.float32)
    for t, (m, jc) in enumerate(RAGGED_LIST):
        q = 512 * m + np.arange(512)[None, :]
        j = 128 * jc + np.arange(128)[:, None]
        mask[t] = np.where(3 * j > q, 0.0, 1.0).astype(np.float32)
    return mask


def make_sel() -> np.ndarray:
    m = np.arange(2048)
    k_of_m = 2 * (m // 128) + (m % 128) // 64
    sel = (np.arange(32)[:, None] == k_of_m[None, :]).astype(np.float32)
    return sel


def prep_inputs(q, k, Wq, Wk, Wv, Wo, conv_w):
    """Returns list of 8 in_maps (core c = 2b + g)."""
    import ml_dtypes
    bf = ml_dtypes.bfloat16
    Wc = np.ascontiguousarray(conv_w.transpose(2, 1, 0).reshape(KK, D))
    mask = make_mask()
    sel = make_sel()
    in_maps = []
    for c in range(8):
        b, g = c // 2, c % 2
        sl = slice(DHALF * g, DHALF * (g + 1))
        # kc-channel contraction order: [own; peer] for RS, canonical for AllGather
        if USE_RS:
            perm = np.r_[np.arange(DHALF * g, DHALF * (g + 1)),
                         np.arange(DHALF * (1 - g), DHALF * (2 - g))]
        else:
            perm = np.arange(D)
        sab = np.zeros((128, 2), dtype=np.float32)
        sab[:, 0] = float(g)        # slot A scale
        sab[:, 1] = float(1 - g)    # slot B scale
        in_maps.append({
            "qT": np.ascontiguousarray(q[b].T).astype(bf),
            "kT": np.ascontiguousarray(k[b].T).astype(bf),
            "WcH": np.ascontiguousarray(Wc[:, sl]).astype(bf),
            "WqT": np.ascontiguousarray(Wq[sl, :].T).astype(bf),
            "WkT": np.ascontiguousarray(Wk[sl, :].T[perm, :]).astype(bf),
            "WvT": np.ascontiguousarray(Wv[sl, :].T[perm, :]).astype(bf),
            "WoT": np.ascontiguousarray(Wo[:, sl].T).astype(bf),
            "MASK": mask.astype(bf),
            "K0H": np.ascontiguousarray(k[b, 0, sl].reshape(DHALF, 1)).astype(bf),
            "SAB": sab,
            "SEL": sel,
        })
    return in_maps


def postprocess(results, bo):
    out = np.zeros((B, T, D), dtype=np.float32)
    for b in range(B):
        out[b] = (np.asarray(results[2 * b]["out_p"], dtype=np.float32)
                  + np.asarray(results[2 * b + 1]["out_p"], dtype=np.float32)
                  + bo[None, :])
    return out


_CACHED_NC = None


def kernel(q, k, v, Wq, Wk, Wv, Wo, bo, conv_w):
    """Full-input entry point. v is unused by the reference computation
    (V is replaced by the conv-compressed K)."""
    global _CACHED_NC
    from concourse.bass_utils import run_bass_kernel_spmd

    q = np.asarray(q, dtype=np.float32)
    k = np.asarray(k, dtype=np.float32)
    Wq = np.asarray(Wq, dtype=np.float32)
    Wk = np.asarray(Wk, dtype=np.float32)
    Wv = np.asarray(Wv, dtype=np.float32)
    Wo = np.asarray(Wo, dtype=np.float32)
    bo = np.asarray(bo, dtype=np.float32)
    conv_w = np.asarray(conv_w, dtype=np.float32)

    in_maps = prep_inputs(q, k, Wq, Wk, Wv, Wo, conv_w)
    if _CACHED_NC is None:
        nc = build_nc()
        nc.finalize()
        _CACHED_NC = nc
    res = run_bass_kernel_spmd(_CACHED_NC, in_maps, list(range(8)))
    return postprocess(res.results, bo)
